# revision 19
# baseline (speedup 1.0000x reference)
"""Tensor-parallel GQA attention block (qk-norm + partial RoPE + sigmoid gate)
for 8 Trainium2 NeuronCores.

The wall-clock cost of this problem is dominated by host->device transfer over
the axon tunnel (~30-90 MB/s, large fixed cost per array), not device compute
(~0.1 s).  The kernel is organized around minimizing shipped bytes and array
count:

  - ONE packed input blob per core (~5.3 MB) holding: a 512-token fp16 shard
    of X^T plus 1/8 of the RoPE table (both AllGathered on device), Wk or Wv
    for the pair's kv head (pairwise-AllGathered: even core ships Wk, odd
    ships Wv), this core's private wo fp16 columns, the gate AND query
    weights as per-column-scaled int8 (gate dequant folds into the PSUM->SBUF
    copy; query dequant is one broadcast multiply before the RMS-norm), and
    the qk-norm / dequant-scale rows (broadcast on device via K=1 matmuls).
    Total upload 38 MB vs 451 MB for the naive full-replication scheme.
  - The causal mask is generated on device with gpsimd.affine_select (no mask
    upload).  If the input mask is not exactly causal, a fallback program
    variant ships exp(mask) as a second fp16 input (multiplicative-mask
    formulation; degenerate for fully-masked query rows, as in the original).
  - Identity / ones / eps constants are generated on device.
  - Output is int8 with per-row-block scales embedded as 4 bitcast uint8
    columns (absmax/127 quantization, adds ~4e-3 rel err vs the 2e-2 budget);
    halves both the donated zero-output upload and the D2H download.
  - jax persistent compilation cache enabled with min_compile_time 0 so the
    walrus-compiled executable is reused across calls/processes (the per-call
    rebuild otherwise costs ~0.4 s).
  - phases 1 and 3 run as tc.For_i hardware loops (program-size cost is
    ~25-47 us per EMITTED instruction; looping cuts ~2100 emitted
    instructions).  Phase 2 stays unrolled: its loop variants cost more in
    per-iteration all-engine barriers than they save.

Compute sharding: 16 query heads / 8 cores = 2 q-heads per core with the
matching KV head per pair; per-core attention with transposed scores and
exp-without-max softmax (safe: qk-norm bounds |s| <= sqrt(HD)); gated head
outputs AllGathered on device; each core computes a 256-column shard of the
output projection.
"""

import time

import numpy as np
from contextlib import ExitStack

try:  # persistent XLA/NEFF cache across processes (best effort)
    import jax as _jax
    _jax.config.update("jax_compilation_cache_dir", "/tmp/jax_kernel_cache")
    _jax.config.update("jax_persistent_cache_min_compile_time_secs", 0.0)
except Exception:
    pass

import concourse.bacc as bacc
import concourse.tile as tile
from concourse import mybir
from concourse import masks as cmasks
from concourse.bass_utils import run_bass_kernel_spmd

F32 = mybir.dt.float32
F32R = mybir.dt.float32r
F16 = mybir.dt.float16
U8 = mybir.dt.uint8
I8 = mybir.dt.int8

B, S, HID = 2, 2048, 2048
NH, NKV, HD = 16, 4, 128
ROT, THETA, EPS = 32, 10000.0, 1e-6
NCORES = 8
T = B * S                       # 4096 tokens
P = 128                         # partitions
KT = HID // P                   # 16 contraction tiles
QT = S // 512                   # 4 q-tiles of 512 per batch
SKT = S // P                    # 16 k-tiles of 128 per batch
H_LOC = NH // NCORES            # 2 q heads per core
CW = H_LOC * HD                 # 256 local head columns
TB = T // 512                   # 8 token blocks of 512 == NCORES

# blob layout (fp16 elements)
XSZ = HID * 512                 # X^T shard [HID, 512]
CSSH = 256 * 64                 # this core's 1/8 of the c32|s32 table
SHSZ = XSZ + CSSH               # region riding the global AllGather
KVSZ = HID * HD                 # Wk (even cores) or Wv (odd) for the pair's kv head
PRIVSZ = HID * 256              # wo fp16
G8SZ = HID * CW // 2            # gate weights int8, in fp16 slots
Q8SZ = HID * CW // 2            # wq weights int8, in fp16 slots
OKV, OPRIV = SHSZ, SHSZ + KVSZ
OG8 = OPRIV + PRIVSZ
OQ8 = OG8 + G8SZ
OQKW = OQ8 + Q8SZ
OGSC = OQKW + 384               # gate per-column scales fp16
OQSC = OGSC + CW                # q dequant scales (256) + ones for k (128)
NBLOB = OQSC + 384

FREE, MIXED, MASKED = 0, 1, 2

_PROGRAM_CACHE = {}
LAST_RUN_SECONDS = None


def _emit(tc, io, cls, use_affine, sim=False, collective=True):
    nc = tc.nc
    blob = io["blob"]
    sh_re = blob[0:SHSZ]                                                 # X + CS shard
    kv_re = blob[OKV:OKV + KVSZ]                                         # Wk or Wv
    wre = (blob[OPRIV:OPRIV + PRIVSZ].rearrange("(h n) -> h n", n=CW)
           .rearrange("(k p) n -> p k n", p=P))                          # [P, KT, 256]
    g8re = (blob[OG8:OG8 + G8SZ].bitcast(I8)
            .rearrange("(h n) -> h n", n=CW)
            .rearrange("(k p) n -> p k n", p=P))                         # [P, KT, 256] i8
    q8re = (blob[OQ8:OQ8 + Q8SZ].bitcast(I8)
            .rearrange("(h n) -> h n", n=CW)
            .rearrange("(k p) n -> p k n", p=P))                         # [P, KT, 256] i8
    qkwre = blob[OQKW:OQKW + 384].rearrange("(a n) -> a n", a=1)         # [1, 384]
    gscre = blob[OGSC:OGSC + CW].rearrange("(h p) -> p h", p=P)          # [P, 2]
    qscre = blob[OQSC:NBLOB].rearrange("(a n) -> a n", a=1)              # [1, 384]

    with ExitStack() as ctx:
        consts = ctx.enter_context(tc.tile_pool(name="consts", bufs=1))

        wqk_sb = consts.tile([P, KT, 384], F16)
        wq8_sb = consts.tile([P, KT, CW], I8)
        nc.sync.dma_start(out=wq8_sb, in_=q8re)
        nc.vector.tensor_copy(wqk_sb[:, :, 0:256], wq8_sb[:])
        wo_sb = consts.tile([P, KT, CW], F16)
        nc.sync.dma_start(out=wo_sb, in_=wre)
        wg8_sb = consts.tile([P, KT, CW], I8)
        nc.sync.dma_start(out=wg8_sb, in_=g8re)
        wg_sb = consts.tile([P, KT, CW], F16)
        nc.vector.tensor_copy(wg_sb[:], wg8_sb[:])
        gsc16 = consts.tile([P, H_LOC], F16)
        nc.sync.dma_start(out=gsc16, in_=gscre)
        gsc_sb = consts.tile([P, H_LOC], F32)
        nc.vector.tensor_copy(gsc_sb[:], gsc16[:])
        qkwrow_sb = consts.tile([1, 384], F16)
        nc.sync.dma_start(out=qkwrow_sb, in_=qkwre)
        qscrow_sb = consts.tile([1, 384], F16)
        nc.sync.dma_start(out=qscrow_sb, in_=qscre)
        ones_row = consts.tile([1, P], F16)
        nc.vector.memset(ones_row[:], 1.0)
        c_all = consts.tile([P, 16, 96], F16)
        s_all = consts.tile([P, 16, 96], F16)
        ident_sb = consts.tile([P, P], F32)
        cmasks.make_identity(nc, ident_sb[:])
        ones_sb = consts.tile([P, 1], F32)
        nc.vector.memset(ones_sb[:], 1.0)
        onescol_sb = consts.tile([1, P], F32)
        nc.vector.memset(onescol_sb[:], 1.0)
        eps_sb = consts.tile([P, 1], F32)
        nc.vector.memset(eps_sb[:], EPS)
        qkw_sb = consts.tile([P, 384], F16)
        sbc_sb = consts.tile([P, 384], F32)
        with tc.tile_pool(name="ps_init", bufs=2, space="PSUM") as ps_init:
            qkw_ps = ps_init.tile([P, 384], F32, tag="a")
            nc.tensor.matmul(qkw_ps[:], ones_row[:], qkwrow_sb[:],
                             start=True, stop=True)
            nc.any.tensor_copy(qkw_sb[:], qkw_ps[:])
            sbc_ps = ps_init.tile([P, 384], F32, tag="b")
            nc.tensor.matmul(sbc_ps[:], ones_row[:], qscrow_sb[:],
                             start=True, stop=True)
            nc.any.tensor_copy(sbc_sb[:], sbc_ps[:])

        dram = ctx.enter_context(tc.tile_pool(name="dram", bufs=1, space="DRAM"))
        gdram = dram.tile([B, H_LOC, P, S], F16)
        ag_in_sh = dram.tile([SHSZ], F16)
        ag_sh = dram.tile([NCORES, SHSZ], F16, addr_space="Shared")
        kv_in = dram.tile([KVSZ], F16)
        kv_g = dram.tile([2, KVSZ], F16)
        ag_in = dram.tile([CW, T], F16)
        ag_out = dram.tile([NCORES * CW, T], F16, addr_space="Shared")

        # ---------------- input AllGathers -----------------
        # collectives cannot read IO tensors: stage via internal DRAM tiles.
        # global: X token-shard + 1/8 of the RoPE table; pairwise: Wk from the
        # even core + Wv from the odd core of each kv-head pair.
        nc.sync.dma_start(out=ag_in_sh, in_=sh_re)
        nc.sync.dma_start(out=kv_in, in_=kv_re)
        if sim or not collective:
            nc.sync.dma_start(out=ag_sh[0], in_=ag_in_sh[:])
            nc.sync.dma_start(out=kv_g[0], in_=kv_in[:])
        else:
            nc.gpsimd.collective_compute(
                "AllGather",
                mybir.AluOpType.bypass,
                ins=[ag_in_sh.opt()],
                outs=[ag_sh.opt()],
                replica_groups=[list(range(NCORES))],
            )
            nc.gpsimd.collective_compute(
                "AllGather",
                mybir.AluOpType.bypass,
                ins=[kv_in.opt()],
                outs=[kv_g.opt()],
                replica_groups=[[2 * j, 2 * j + 1] for j in range(NCORES // 2)],
            )
        nc.sync.dma_start(
            out=wqk_sb[:, :, 256:384],
            in_=kv_g[0, :].rearrange("(h n) -> h n", n=HD)
            .rearrange("(k p) n -> p k n", p=P),
        )
        wv_sb = consts.tile([P, KT, HD], F16)
        nc.sync.dma_start(
            out=wv_sb,
            in_=kv_g[1, :].rearrange("(h n) -> h n", n=HD)
            .rearrange("(k p) n -> p k n", p=P),
        )
        for g in range(16):
            seg, lr = g // 2, (g % 2) * P
            v = (ag_sh[seg, XSZ + lr * 64: XSZ + (lr + P) * 64]
                 .rearrange("(p c) -> p c", c=64))
            nc.sync.dma_start(out=c_all[:, g, 0:32], in_=v[:, 0:32])
            nc.sync.dma_start(out=s_all[:, g, 0:32], in_=v[:, 32:64])
        for rep in (1, 2):
            nc.vector.tensor_copy(c_all[:, :, rep * 32:(rep + 1) * 32],
                                  c_all[:, :, 0:32])
            nc.vector.tensor_copy(s_all[:, :, rep * 32:(rep + 1) * 32],
                                  s_all[:, :, 0:32])

        acts = ctx.enter_context(tc.tile_pool(name="acts", bufs=1))
        qTb = {}
        kT_ = {}
        v_ = {}
        for b in range(B):
            qTb[b] = acts.tile([P, H_LOC, S], F16, tag=f"qT{b}", name=f"qT{b}")
            kT_[b] = acts.tile([P, S], F16, tag=f"kT{b}", name=f"kT{b}")
            v_[b] = acts.tile([P, S], F32, tag=f"v{b}", name=f"v{b}")

        # ---------------- Phase 1: projections -----------------
        with ExitStack() as p1:
            xtp = p1.enter_context(tc.tile_pool(name="xt", bufs=22))
            wkp = p1.enter_context(tc.tile_pool(name="p1sb", bufs=3))
            ps_qk = p1.enter_context(tc.tile_pool(name="ps_qk", bufs=3, space="PSUM"))
            ps_t = p1.enter_context(tc.tile_pool(name="ps_t", bufs=2, space="PSUM"))
            ps_vg = p1.enter_context(tc.tile_pool(name="ps_vg", bufs=1, space="PSUM"))

            agsx = ag_sh[:, 0:XSZ].rearrange("tb (h t) -> tb h t", t=512)
            gdv = gdram[:].rearrange("b h p (t n) -> b h p t n", n=512)
            for b in range(B):
              with tc.For_i(0, QT, 1) as ti:
                    tb = b * QT + ti
                    xT = []
                    for kt in range(KT):
                        xt_t = xtp.tile([P, 512], F16, tag="xT")
                        nc.sync.dma_start(
                            out=xt_t, in_=agsx[tb, kt * P:(kt + 1) * P, :]
                        )
                        xT.append(xt_t)

                    # V^T and gate^T head-major, accumulate over kt
                    v_ps = ps_vg.tile([P, 512], F32, tag="v_ps")
                    g_ps = [ps_vg.tile([P, 512], F32, tag=f"g{h}_ps", name=f"g{h}_ps")
                            for h in range(H_LOC)]
                    for kt in range(KT):
                        st_flags = dict(start=(kt == 0), stop=(kt == KT - 1))
                        nc.tensor.matmul(v_ps[:], wv_sb[:, kt, :], xT[kt][:],
                                         **st_flags)
                        for h in range(H_LOC):
                            nc.tensor.matmul(
                                g_ps[h][:], wg_sb[:, kt, h * HD:(h + 1) * HD],
                                xT[kt][:], **st_flags
                            )
                    vts = wkp.tile([P, 512], F32, tag="vts")
                    nc.any.tensor_copy(vts[:], v_ps[:])
                    vv = v_[b][:].rearrange("p (t n) -> p t n", n=512)
                    for sub in range(4):
                        tp = ps_t.tile([P, P], F32, tag="tp")
                        nc.tensor.transpose(tp[:], vts[:, sub * P:(sub + 1) * P],
                                            ident_sb[:])
                        nc.any.tensor_copy(vv[:, ti, sub * P:(sub + 1) * P], tp[:])
                    for h in range(H_LOC):
                        gts = wkp.tile([P, 512], F16, tag=f"gts{h}")
                        nc.vector.tensor_scalar_mul(
                            out=gts[:], in0=g_ps[h][:],
                            scalar1=gsc_sb[:, h:h + 1])
                        nc.sync.dma_start(
                            out=gdv[b, h, :, ti, :], in_=gts
                        )

                    # Q/K token-major per 128-token sub-tile
                    cav = c_all[:].rearrange("p (t s) n -> p t s n", s=4)
                    sav = s_all[:].rearrange("p (t s) n -> p t s n", s=4)
                    for st in range(4):
                        qk_ps = ps_qk.tile([P, 384], F32, tag="qk_ps")
                        for kt in range(KT):
                            nc.tensor.matmul(
                                qk_ps[:], xT[kt][:, st * P:(st + 1) * P],
                                wqk_sb[:, kt, :],
                                start=(kt == 0), stop=(kt == KT - 1),
                            )
                        # dequant int8-sourced q columns (k columns scale 1.0)
                        qkd = wkp.tile([P, 384], F32, tag="qkd")
                        nc.vector.tensor_mul(qkd[:], qk_ps[:], sbc_sb[:])

                        # RMS norm over each 128-col head block
                        junk = wkp.tile([P, P], F32, tag="junk")
                        ssq = wkp.tile([P, 3], F32, tag="ssq")
                        for blk in range(3):
                            nc.scalar.activation(
                                out=junk[:], in_=qkd[:, blk * P:(blk + 1) * P],
                                func=mybir.ActivationFunctionType.Square,
                                accum_out=ssq[:, blk:blk + 1],
                            )
                        rstd = wkp.tile([P, 3], F32, tag="rstd")
                        nc.scalar.activation(
                            out=rstd[:], in_=ssq[:],
                            func=mybir.ActivationFunctionType.Sqrt,
                            bias=eps_sb[:], scale=1.0 / HD,
                        )
                        nc.vector.reciprocal(rstd[:], rstd[:])
                        qkn = wkp.tile([P, 384], F32, tag="qkn")
                        for blk in range(3):
                            nc.vector.tensor_scalar_mul(
                                out=qkn[:, blk * P:(blk + 1) * P],
                                in0=qkd[:, blk * P:(blk + 1) * P],
                                scalar1=rstd[:, blk:blk + 1],
                            )
                        nc.vector.tensor_mul(qkn[:], qkn[:], qkw_sb[:])

                        # RoPE on cols [0:32] of each block
                        qkn3 = qkn[:].rearrange("p (b n) -> p b n", b=3)
                        c3v = cav[:, ti, st, :].rearrange("p (b n) -> p b n", b=3)
                        s3v = sav[:, ti, st, :].rearrange("p (b n) -> p b n", b=3)
                        shuf = wkp.tile([P, 3, ROT], F32, tag="shuf")
                        half = ROT // 2
                        nc.vector.tensor_copy(shuf[:, :, 0:half], qkn3[:, :, half:ROT])
                        nc.vector.tensor_copy(shuf[:, :, half:ROT], qkn3[:, :, 0:half])
                        nc.vector.tensor_mul(shuf[:], shuf[:], s3v)
                        rot = wkp.tile([P, 3, ROT], F32, tag="rot")
                        nc.vector.tensor_mul(rot[:], qkn3[:, :, 0:ROT], c3v)
                        nc.vector.tensor_add(qkn3[:, :, 0:ROT], rot[:], shuf[:])

                        # transpose to head-major
                        qv = qTb[b][:].rearrange("p h (t s n) -> p h t s n",
                                                 t=QT, s=4)
                        kv2 = kT_[b][:].rearrange("p (t s n) -> p t s n",
                                                  t=QT, s=4)
                        for blk in range(3):
                            tp = ps_t.tile([P, P], F32, tag="tp")
                            nc.tensor.transpose(
                                tp[:], qkn[:, blk * P:(blk + 1) * P], ident_sb[:]
                            )
                            if blk < 2:
                                nc.any.tensor_copy(qv[:, blk, ti, st, :], tp[:])
                            else:
                                nc.any.tensor_copy(kv2[:, ti, st, :], tp[:])

        # ---------------- Phase 2: attention -----------------
        with ExitStack() as p2:
            mkp = p2.enter_context(tc.tile_pool(name="mask", bufs=2))
            exp_p = p2.enter_context(tc.tile_pool(name="expp", bufs=4))
            ep_p = p2.enter_context(tc.tile_pool(name="epp", bufs=3))
            ps_sc = p2.enter_context(tc.tile_pool(name="ps_sc", bufs=3, space="PSUM"))
            ps_at = p2.enter_context(tc.tile_pool(name="ps_at", bufs=2, space="PSUM"))
            ps_se = p2.enter_context(tc.tile_pool(name="ps_se", bufs=2, space="PSUM"))
            ps_rb = p2.enter_context(tc.tile_pool(name="ps_rb", bufs=1, space="PSUM"))

            for qt in range(QT):
                ixs = [kt for kt in range(SKT) if cls[qt][kt] != MASKED]
                mk = {}
                if not use_affine:
                    for kt in ixs:
                        if cls[qt][kt] == MIXED:
                            m = mkp.tile([P, 512], F16, tag=f"mk{kt}")
                            nc.sync.dma_start(
                                out=m,
                                in_=io["maskexp"][kt * P:(kt + 1) * P,
                                                  qt * 512:(qt + 1) * 512],
                            )
                            mk[kt] = m
                for b in range(B):
                    for h in range(H_LOC):
                        at_ps = ps_at.tile([P, 512], F32, tag="at")
                        se_ps = ps_se.tile([1, 512], F32, tag="se")
                        for kt in ixs:
                            sc = ps_sc.tile([P, 512], F32, tag="sc")
                            nc.tensor.matmul(
                                sc[:], kT_[b][:, kt * P:(kt + 1) * P],
                                qTb[b][:, h, qt * 512:(qt + 1) * 512],
                                start=True, stop=True,
                            )
                            ex = exp_p.tile([P, 512], F32, tag="ex")
                            nc.scalar.activation(
                                out=ex[:], in_=sc[:],
                                func=mybir.ActivationFunctionType.Exp,
                            )
                            if cls[qt][kt] == MIXED:
                                if use_affine:
                                    # keep where q_abs >= k_abs, else 0
                                    nc.gpsimd.affine_select(
                                        out=ex[:], in_=ex[:],
                                        pattern=[[1, 512]],
                                        compare_op=mybir.AluOpType.is_ge,
                                        fill=0.0,
                                        base=qt * 512 - kt * P,
                                        channel_multiplier=-1,
                                    )
                                else:
                                    nc.vector.tensor_mul(ex[:], ex[:], mk[kt][:])
                            flags = dict(start=(kt == ixs[0]), stop=(kt == ixs[-1]))
                            nc.tensor.matmul(
                                at_ps[:], v_[b][:, kt * P:(kt + 1) * P], ex[:], **flags
                            )
                            nc.tensor.matmul(se_ps[:], ones_sb[:], ex[:], **flags)

                        rec = ep_p.tile([1, 512], F32, tag="rec")
                        nc.vector.reciprocal(rec[:], se_ps[:])
                        rb_ps = ps_rb.tile([P, 512], F32, tag="rb")
                        nc.tensor.matmul(rb_ps[:], onescol_sb[:], rec[:],
                                         start=True, stop=True)
                        rbs = ep_p.tile([P, 512], F32, tag="rbs")
                        nc.any.tensor_copy(rbs[:], rb_ps[:])
                        gt = ep_p.tile([P, 512], F16, tag="gt")
                        nc.sync.dma_start(
                            out=gt, in_=gdram[b, h, :, qt * 512:(qt + 1) * 512]
                        )
                        sig = ep_p.tile([P, 512], F32, tag="sig")
                        nc.scalar.activation(
                            out=sig[:], in_=gt[:],
                            func=mybir.ActivationFunctionType.Sigmoid,
                        )
                        tmp = ep_p.tile([P, 512], F32, tag="tmp")
                        nc.vector.tensor_mul(tmp[:], at_ps[:], rbs[:])
                        ag = ep_p.tile([P, 512], F16, tag="ag")
                        nc.vector.tensor_mul(ag[:], tmp[:], sig[:])
                        nc.sync.dma_start(
                            out=ag_in[h * P:(h + 1) * P,
                                      b * S + qt * 512: b * S + (qt + 1) * 512],
                            in_=ag,
                        )

        # ---------------- AllGather -----------------
        if sim or not collective:
            # stand-in (no collectives in TimelineSim / isolation bench)
            nc.sync.dma_start(out=ag_out[0:CW, :], in_=ag_in[:])
        else:
            nc.gpsimd.collective_compute(
                "AllGather",
                mybir.AluOpType.bypass,
                ins=[ag_in.opt()],
                outs=[ag_out.opt()],
                replica_groups=[list(range(NCORES))],
            )

        # ---------------- Phase 3: output projection -----------------
        # hardware loop over the 8 output tiles: the body is emitted once
        # (program-size cost is per EMITTED instruction, ~47us each)
        agv = ag_out[:].rearrange("r (tt t) -> r tt t", tt=TB)
        outv = io["out"].rearrange("(tt r) c -> tt r c", tt=TB)
        with ExitStack() as p3:
            x2p = p3.enter_context(tc.tile_pool(name="x2", bufs=8))
            o_p = p3.enter_context(tc.tile_pool(name="osb", bufs=4))
            ps_o = p3.enter_context(tc.tile_pool(name="ps_o", bufs=1, space="PSUM"))

            with tc.For_i(0, TB, 1) as iv:
                o_ps = [ps_o.tile([P, CW], F32, tag=f"o{st}", name=f"o{st}_ps")
                        for st in range(4)]
                for kt in range(KT):
                    x2 = x2p.tile([P, 512], F16, tag="x2")
                    nc.sync.dma_start(
                        out=x2, in_=agv[kt * P:(kt + 1) * P, iv, :],
                    )
                    for st in range(4):
                        nc.tensor.matmul(
                            o_ps[st][:], x2[:, st * P:(st + 1) * P],
                            wo_sb[:, kt, :],
                            start=(kt == 0), stop=(kt == KT - 1),
                        )
                for st in range(4):
                    am = o_p.tile([P, 1], F32, tag="am")
                    nc.vector.tensor_reduce(
                        out=am[:], in_=o_ps[st][:], axis=mybir.AxisListType.X,
                        op=mybir.AluOpType.max, apply_absolute_value=True)
                    nc.vector.tensor_scalar_max(out=am[:], in0=am[:], scalar1=1e-30)
                    s = o_p.tile([P, 1], F32, tag="s")
                    nc.vector.reciprocal(s[:], am[:])
                    nc.vector.tensor_scalar_mul(out=s[:], in0=s[:], scalar1=127.0)
                    q = o_p.tile([P, CW + 4], U8, tag="q")
                    nc.vector.tensor_scalar(
                        out=q[:, 0:CW], in0=o_ps[st][:], scalar1=s[:],
                        scalar2=128.0, op0=mybir.AluOpType.mult,
                        op1=mybir.AluOpType.add)
                    nc.vector.tensor_copy(q[:, CW:CW + 4], am[:].bitcast(U8))
                    nc.sync.dma_start(out=outv[iv, st * P:(st + 1) * P, :], in_=q)


def _build_program(cls, use_affine, sim=False, collective=True):
    nc = bacc.Bacc("TRN2", target_bir_lowering=False,
                   num_devices=1 if sim else NCORES)
    io = {
        "blob": nc.dram_tensor("blob", [NBLOB], F16, kind="ExternalInput").ap(),
        "out": nc.dram_tensor("out", [T, CW + 4], U8, kind="ExternalOutput").ap(),
    }
    if not use_affine:
        io["maskexp"] = nc.dram_tensor("maskexp", [S, S], F16,
                                       kind="ExternalInput").ap()
    with tile.TileContext(nc) as tc:
        _emit(tc, io, cls, use_affine, sim=sim, collective=collective)
    nc.compile()
    return nc


def _host_prep(hidden_states, attention_mask, Wq, Wk, Wv, Wo, q_norm_w, k_norm_w):
    """Returns (cls, use_affine, cls_key, in_maps)."""
    hidden_states = np.asarray(hidden_states, dtype=np.float32)
    attention_mask = np.asarray(attention_mask, dtype=np.float32)
    Wq = np.asarray(Wq, dtype=np.float32)
    Wk = np.asarray(Wk, dtype=np.float32)
    Wv = np.asarray(Wv, dtype=np.float32)
    Wo = np.asarray(Wo, dtype=np.float32)
    q_norm_w = np.asarray(q_norm_w, dtype=np.float32)
    k_norm_w = np.asarray(k_norm_w, dtype=np.float32)

    # ---- mask handling: exact-causal fast path, general fallback ----
    mask = attention_mask[0, 0]
    tril = np.tri(S, dtype=bool)
    use_affine = bool(np.array_equal(mask == 0.0, tril)
                      and np.all(np.where(tril, -200.0, mask) <= -103.0))
    maskexp_f16 = None
    if use_affine:
        cls = [[FREE if kt < 4 * qt else (MIXED if kt <= 4 * qt + 3 else MASKED)
                for kt in range(SKT)] for qt in range(QT)]
    else:
        with np.errstate(over="ignore", under="ignore"):
            me = np.exp(mask)                                   # [q, k]
        maskexpT = np.ascontiguousarray(me.T)                   # [k, q]
        cls = []
        for qt in range(QT):
            row = []
            for kt in range(SKT):
                blk = maskexpT[kt * P:(kt + 1) * P, qt * 512:(qt + 1) * 512]
                if np.all(blk == 1.0):
                    row.append(FREE)
                elif np.all(blk == 0.0):
                    row.append(MASKED)
                else:
                    row.append(MIXED)
            cls.append(row)
        maskexp_f16 = maskexpT.astype(np.float16)
    cls_key = (use_affine, tuple(tuple(r) for r in cls))

    # ---- host packing: one fp16 blob per core ----
    xT16 = hidden_states.reshape(T, HID).T.astype(np.float16)   # [HID, T]

    inv = THETA ** (-np.arange(0, ROT, 2, dtype=np.float64) / ROT)      # [16]
    fr = np.arange(S, dtype=np.float64)[:, None] * inv[None, :]         # [S, 16]
    cos16 = np.cos(fr).astype(np.float32)
    sin16 = np.sin(fr).astype(np.float32)
    c32 = np.concatenate([cos16, cos16], axis=1)                        # [S, 32]
    s32 = np.concatenate([-sin16, sin16], axis=1)                       # [S, 32]
    cs64 = np.concatenate([c32, s32], axis=1).astype(np.float16).ravel()

    qs = 1.0 / np.sqrt(HD)
    qkw_row = np.concatenate([np.tile(q_norm_w * qs, 2), k_norm_w])     # [384]

    cs2d = cs64.reshape(S, 64)
    qkw16 = qkw_row.astype(np.float16)
    in_maps = []
    for c in range(NCORES):
        j = c // 2  # kv head
        w16 = np.ascontiguousarray(Wo[:, CW * c:CW * (c + 1)]).astype(np.float16)
        g = Wq[:, NH * HD + CW * c: NH * HD + CW * (c + 1)]
        gsc = np.maximum(np.abs(g).max(axis=0), 1e-20) / 127.0
        g8 = np.round(g / gsc[None, :]).astype(np.int8)
        wq = Wq[:, CW * c:CW * (c + 1)]
        qsc = np.maximum(np.abs(wq).max(axis=0), 1e-20) / 127.0
        q8 = np.round(wq / qsc[None, :]).astype(np.int8)
        qscrow = np.concatenate([qsc, np.ones(P, np.float32)]).astype(np.float16)
        kv_src = Wk if c % 2 == 0 else Wv
        blob = np.empty(NBLOB, np.float16)
        blob[0:XSZ] = xT16[:, c * 512:(c + 1) * 512].ravel()
        blob[XSZ:SHSZ] = cs2d[c * 256:(c + 1) * 256].ravel()
        blob[OKV:OPRIV] = kv_src[:, HD * j:HD * (j + 1)].astype(np.float16).ravel()
        blob[OPRIV:OG8] = w16.ravel()
        blob[OG8:OQ8].view(np.int8)[:] = g8.ravel()
        blob[OQ8:OQKW].view(np.int8)[:] = q8.ravel()
        blob[OQKW:OQKW + 384] = qkw16
        blob[OGSC:OGSC + CW] = gsc.astype(np.float16)
        blob[OQSC:NBLOB] = qscrow
        m = {"blob": blob}
        if not use_affine:
            m["maskexp"] = maskexp_f16
        in_maps.append(m)

    return cls, use_affine, cls_key, in_maps


def kernel(hidden_states, attention_mask, Wq, Wk, Wv, Wo, q_norm_w, k_norm_w):
    global LAST_RUN_SECONDS
    cls, use_affine, cls_key, in_maps = _host_prep(
        hidden_states, attention_mask, Wq, Wk, Wv, Wo, q_norm_w, k_norm_w)
    if cls_key not in _PROGRAM_CACHE:
        _PROGRAM_CACHE[cls_key] = _build_program(cls, use_affine)
    nc = _PROGRAM_CACHE[cls_key]

    t0 = time.perf_counter()
    res = run_bass_kernel_spmd(nc, in_maps, core_ids=list(range(NCORES)))
    LAST_RUN_SECONDS = time.perf_counter() - t0

    out = np.empty((T, NH * HD), dtype=np.float32)
    for c in range(NCORES):
        r = res.results[c]["out"]
        scale = (r[:, CW:CW + 4].copy().view(np.float32)[:, 0] / 127.0)
        out[:, CW * c:CW * (c + 1)] = ((r[:, 0:CW].astype(np.float32) - 128.0)
                                       * scale[:, None])
    return out.reshape(B, S, NH * HD)
